# revision 55
# baseline (speedup 1.0000x reference)
"""NT-Xent loss kernel for Trainium2, 8-core SPMD.

Math (matches the reference exactly):
  reps = concat(z_i, z_j)                       [2B, C], B=4096, C=128
  rhat = reps / ||reps||                        (row L2 normalize)
  sim  = rhat @ rhat.T                          [2B, 2B]  (never materialized)
  pos_r = sim[r, (r+B) mod 2B]
  loss = mean_r( ln(S_r - e^2 + e^{2 pos_r}) - 2 pos_r ),
  S_r = sum_c exp(2 rhat_r . rhat_c)   (diag contributes e^{2|rhat_r|^2} ~ e^2)

v7 design (v5 fp32 baseline 122us -> 90us):
  - host: rotate rows by k*1024 per core, cast bf16 and pre-tile to
    [128p, 8192] so each core's query block IS key-tiles 0:8 and its
    positive block is tiles 32:40: no separate q/p DMAs (2MB total in,
    one descriptor per partition), exp scale is the CONSTANT 2.0 (q is
    pre-normalized), the masked diagonal is the constant e^2, and all
    cores run an identical program.
  - bf16 matmul operands (PE 1 cyc/col); fp32 loss math; per-element
    bf16 rounding averages out in the 8192-term rowsums (measured
    ~1e-6 end-to-end vs the fp32 reference).
  - key transposes via the DMA xbar (dma_start_transpose, ONE instr
    per column group on the otherwise-idle Sync queue): PE/PSUM/DVE
    stay out of the transpose path; PSUM double-buffers matmul+exp.
  - rsqrt for row norms on the DVE (magic seed + one Newton step, all
    int/f32 bitcast ALU): no cross-engine hop in the head chain, and
    ScalarE runs nothing but the exps.
  - ramped column groups [4,12,16,16,16]: first exp fires ~13us after
    boot on 4 tiles instead of waiting for a full 16-tile group.
  - per m-step, a 512-col sub-span of the sim row is exponentiated on
    the DVE with the Schraudolph int-bitcast exp (tensor_scalar
    mult+add -> int32, reduce over the f32-bitcast view) on every step
    where the DVE is not busy with key prep; the sawtooth error is
    bias-tuned (C=0.051) to cancel in the rowsum. ScalarE handles the
    remaining 1536 cols with accum_out rowsums.
  - the [128,1] per-partition partial is collapsed on GpSimd
    (partition_all_reduce) so the output DMA is one descriptor.

Engine occupancy at 90us: Scalar ~76us (exp roofline ~55us + per-instr
overhead + READ_ACCUMULATOR), DVE ~71us, PE ~40us. Fixed costs: ~7us
NEFF boot, ~6us end barrier + teardown.
"""

import os

import numpy as np
import ml_dtypes

import concourse.bacc as bacc
import concourse.bass as bass
import concourse.bass_isa as bass_isa
import concourse.mybir as mybir
from concourse.bass_utils import run_bass_kernel_spmd
from concourse.tile import TileContext

F32 = mybir.dt.float32
BF16 = mybir.dt.bfloat16
AF = mybir.ActivationFunctionType
ALU = mybir.AluOpType
AX = mybir.AxisListType

B = 4096
C = 128
TWOB = 2 * B
N_CORES = 8
M_LOCAL = TWOB // N_CORES   # 1024 query rows per core
MT = M_LOCAL // 128         # 8 m-tiles of 128 queries
NT = TWOB // 128            # 64 key tiles
GSIZE = [4, 12, 16, 16, 16]     # key tiles per column group (ramp-up)
GSTART = [0, 4, 16, 32, 48]
NG = len(GSIZE)
E2 = float(np.exp(2.0))
MAGIC = 0x5F375A86              # Lomont rsqrt seed
# Schraudolph fast-exp: bits(exp(2x)) ~ int(SCH_A*x + SCH_B); C=0.051
# zero-means the sawtooth error over each row's 512-key sub-span sum
SCH_A = float(2.0**24 * 1.4426950408889634)
SCH_B = float(2.0**23 * (127.0 - 0.051))
SCH_G = (1, 2, 3, 4)            # groups eligible for a DVE sub-span
# psum column ranges per (group, m): G0 rotates FOUR 512-wide sub-slots
# (the write-after-read distance then hides the ~1.3us cross-engine sem
# hop); G1 and the steady groups double-buffer.


def ps_slot(g, m):
    if g == 0:
        return (m % 4) * 512
    if g == 1:
        return 1024 if m % 2 == 0 else 2560
    return (m % 2) * 2048


def _patch_act_tables():
    """Leave Exp/Ln only in natural_log_exp_and_others so bacc's greedy
    set chooser emits ONE table load for the whole kernel."""
    if getattr(bacc, "_ntx_act_patched", False):
        return
    orig = bacc.get_activation_tables

    def patched(arch):
        out = {}
        for name, fns in orig(arch).items():
            if name != "natural_log_exp_and_others":
                fns = fns - {AF.Exp, AF.Ln}
            out[name] = fns
        return out

    bacc.get_activation_tables = patched
    bacc._ntx_act_patched = True


def build_bass() -> bass.Bass:
    _patch_act_tables()
    nc = bacc.Bacc()
    keys = nc.dram_tensor("keys", [128, NT * C], BF16, kind="ExternalInput")
    out = nc.dram_tensor("out", [1, 1], F32, kind="ExternalOutput")

    with TileContext(nc) as tc:
        with (
            tc.tile_pool(name="big", bufs=1) as big,
            tc.tile_pool(name="small", bufs=1) as small,
            tc.tile_pool(name="ps", bufs=1, space="PSUM") as psp,
        ):
            kt3 = big.tile([128, NT, C], BF16)
            kn3 = big.tile([128, NT, C], BF16)
            keysT = big.tile([128, NT, C], BF16)
            keysTf = keysT[:].rearrange("p t c -> p (t c)")
            nrm = small.tile([128, NT], F32)
            inv = small.tile([128, NT], F32)
            yseed = small.tile([128, NT], mybir.dt.int32)
            newt = small.tile([128, NT], F32)
            schi = small.tile([128, 512], mybir.dt.int32)
            acc2 = small.tile([128, MT * len(SCH_G)], F32)
            nc.gpsimd.memset(acc2[:], 0.0)
            acc = small.tile([128, MT * NG], F32)
            pos = small.tile([128, MT], F32)
            epos = small.tile([128, MT], F32)
            ps = psp.tile([128, 4096], F32)

            # ---- input DMAs. Each DMA costs ~2.9us of ring time in
            # descriptor processing (128 descs, one per partition)
            # regardless of size, so use as FEW DMAs as possible: the
            # head chunk on the sync ring, the rest on the scalar ring
            # (issued before Scalar has any compute).
            def dma_in(t0, t1, eng):
                eng.dma_start(
                    out=kt3[:, t0:t1, :],
                    in_=keys[:, t0 * C : t1 * C],
                )

            # first group's tiles split across both rings (32 descriptors
            # each) so the critical chain starts ASAP; the rest follows
            nc.scalar.dma_start(out=kt3[64:128, 0:8, :], in_=keys[64:128, 0 : 8 * C])
            dma_in(8, 32, nc.scalar)
            dma_in(32, 64, nc.scalar)
            nc.sync.dma_start(out=kt3[0:64, 0:8, :], in_=keys[0:64, 0 : 8 * C])

            # warm up the PE p-state during the (PE-idle) head: without
            # this the whole ramp runs matmuls at the cold 1.2GHz clock
            # (427ns per 512-col matmul instead of 216ns)
            wdum = big.tile([128, 512], BF16)
            nc.gpsimd.memset(wdum[:], 1.0)
            for _ in range(20):
                nc.tensor.matmul(
                    ps[:, 2048:2560],
                    lhsT=wdum[:, 0:128],
                    rhs=wdum[:],
                    start=True,
                    stop=True,
                )

            def norms(t0, t1):
                n = t1 - t0
                sq = big.tile([128, 8, C], BF16, tag="sq")
                nc.vector.tensor_mul(sq[:, 0:n, :], kt3[:, t0:t1, :], kt3[:, t0:t1, :])
                nc.vector.reduce_sum(nrm[:, t0:t1], sq[:, 0:n, :], axis=AX.X)

            def rsqrt(t0, t1):
                # DVE-only rsqrt (magic seed + one Newton step): keeps the
                # head chain on one engine (no ~1.3us cross-engine sem
                # hops) and Scalar free for the exps.
                nv, iv = nrm[:, t0:t1], inv[:, t0:t1]
                ys, nt = yseed[:, t0:t1], newt[:, t0:t1]
                nc.vector.tensor_scalar(
                    out=ys, in0=nv.bitcast(mybir.dt.int32),
                    scalar1=1, scalar2=None, op0=ALU.logical_shift_right,
                )
                nc.vector.tensor_scalar(
                    out=ys, in0=ys, scalar1=-1, scalar2=MAGIC,
                    op0=ALU.mult, op1=ALU.add,
                )
                y0 = ys.bitcast(F32)
                nc.vector.tensor_mul(nt, y0, y0)          # y^2
                nc.vector.tensor_mul(nt, nt, nv)          # x*y^2
                nc.vector.tensor_scalar(
                    out=nt, in0=nt, scalar1=-0.5, scalar2=1.5,
                    op0=ALU.mult, op1=ALU.add,
                )
                nc.vector.tensor_mul(iv, y0, nt)          # y*(1.5-0.5xy^2)

            def scale(t0, t1):
                for t in range(t0, t1):
                    nc.vector.tensor_scalar_mul(
                        kn3[:, t, :], kt3[:, t, :], inv[:, t : t + 1]
                    )

            def transpose(t0, t1):
                nc.sync.dma_start_transpose(
                    out=keysT[:, t0:t1, :],
                    in_=kn3[:, t0:t1, :].rearrange("p t c -> p (t c)"),
                )

            # ---- head: groups 0 and 1 ready ASAP; strictly serialized
            # mini-chains so the scheduler can't park the critical
            # rsqrt/scale behind later tiles' norms.
            with tc.high_priority():
                norms(0, 4)
                rsqrt(0, 4)
                scale(0, 4)
                transpose(0, 4)
            # tiles 4:8 next: G0's lhsT for m>=4 lives there
            norms(4, 8)      # same DMA as tiles 0:4
            rsqrt(4, 8)
            scale(4, 8)
            transpose(4, 8)
            norms(8, 16)     # first slice of the in(8,32) DMA
            rsqrt(8, 16)
            scale(8, 16)
            transpose(8, 16)

            # ---- main loop: per group, per m-tile: matmuls + fused
            # exp/rowsum; later chunks' norms/scales/transposes emitted
            # where the engines idle.
            for g in range(NG):
                span = GSIZE[g] * 128
                col0 = GSTART[g] * 128
                for m in range(MT):
                    p0 = ps_slot(g, m)
                    psm = ps[:, p0 : p0 + span]
                    for j in range(0, span, 512):
                        nc.tensor.matmul(
                            psm[:, j : j + 512],
                            lhsT=keysTf[:, m * 128 : (m + 1) * 128],
                            rhs=keysTf[:, col0 + j : col0 + j + 512],
                            start=True,
                            stop=True,
                        )
                    # Schraudolph only on steps where DVE isn't doing
                    # key-prep (norms/rsqrt/scale for later groups)
                    sch = g in (3, 4) or (g == 2 and m >= 3)
                    # DVE computes the first 512 cols via Schraudolph
                    # (depends only on matmul j=0, runs alongside the exp)
                    e0 = 512 if sch else 0
                    nc.scalar.activation(
                        psm[:, e0:],
                        psm[:, e0:],
                        AF.Exp,
                        scale=2.0,
                        accum_out=acc[:, m * NG + g : m * NG + g + 1],
                    )
                    if sch:
                        nc.vector.tensor_scalar(
                            out=schi[:, 0:e0], in0=psm[:, 0:e0],
                            scalar1=SCH_A, scalar2=SCH_B,
                            op0=ALU.mult, op1=ALU.add,
                        )
                        c2 = m * len(SCH_G) + (g - SCH_G[0])
                        nc.vector.reduce_sum(
                            acc2[:, c2 : c2 + 1],
                            schi[:, 0:e0].bitcast(F32),
                            axis=AX.X,
                        )
                    if g == 0 and m == 3:
                        norms(16, 24)
                    if g == 0 and m == 5:
                        norms(24, 32)
                    if g == 1 and m == 0:
                        rsqrt(16, 32)
                    if g == 1 and m == 1:
                        scale(16, 32)
                    if g == 1 and m == 2:
                        transpose(16, 32)
                    if g == 1 and m == 3:
                        norms(32, 40)
                    if g == 1 and m == 4:
                        norms(40, 48)
                    if g == 1 and m == 5:
                        rsqrt(32, 48)
                    if g == 1 and m == 6:
                        scale(32, 48)
                    if g == 1 and m == 7:
                        transpose(32, 48)
                    if g == 2 and m == 0:
                        norms(48, 56)
                    if g == 2 and m == 1:
                        norms(56, 64)
                    if g == 2 and m == 2:
                        rsqrt(48, 64)
                    if g == 2 and m == 3:
                        scale(48, 64)
                    if g == 2 and m == 4:
                        transpose(48, 64)
                    if g == 3 and m == 1:
                        # pos_r = qhat_r . phat_r from the normalized tiles
                        prod = big.tile([128, MT, C], F32, tag="prod")
                        nc.vector.tensor_mul(
                            prod[:], kn3[:, 0:MT, :], kn3[:, 32 : 32 + MT, :]
                        )
                        nc.vector.reduce_sum(pos[:], prod[:], axis=AX.X)
                    if g == 3 and m == 3:
                        nc.scalar.activation(epos[:], pos[:], AF.Exp, scale=2.0)

            # ---- finalize: loss_r = ln(S - e^2 + e^{2 pos}) - 2 pos
            S = small.tile([128, MT], F32)
            nc.vector.reduce_sum(
                S[:], acc[:].rearrange("p (m g) -> p m g", g=NG), axis=AX.X
            )
            S2 = small.tile([128, MT], F32)
            nc.vector.reduce_sum(
                S2[:],
                acc2[:].rearrange("p (m g) -> p m g", g=len(SCH_G)),
                axis=AX.X,
            )
            tot = small.tile([128, MT], F32)
            nc.vector.tensor_scalar_add(tot[:], S[:], -E2)
            nc.vector.tensor_add(tot[:], tot[:], S2[:])
            nc.vector.tensor_add(tot[:], tot[:], epos[:])
            nc.scalar.activation(tot[:], tot[:], AF.Ln)
            rowloss = small.tile([128, MT], F32)
            nc.vector.scalar_tensor_tensor(
                out=rowloss[:],
                in0=pos[:],
                scalar=-2.0,
                in1=tot[:],
                op0=ALU.mult,
                op1=ALU.add,
            )
            rsum = small.tile([128, 1], F32)
            nc.vector.reduce_sum(rsum[:], rowloss[:], axis=AX.X)
            # collapse partitions on GpSimd so the out-DMA is a single
            # descriptor (a [128,1] DMA costs ~2.9us of descriptor time)
            nc.gpsimd.partition_all_reduce(
                rsum[:], rsum[:], 128, bass_isa.ReduceOp.add
            )
            nc.sync.dma_start(out=out[:], in_=rsum[0:1, :])

    nc.finalize()
    return nc


_NC_CACHE: bass.Bass | None = None
LAST_RESULTS = None  # BassKernelResults of the last run (for profiling)


def _get_nc() -> bass.Bass:
    global _NC_CACHE
    if _NC_CACHE is None:
        _NC_CACHE = build_bass()
    return _NC_CACHE


def kernel(z_i: np.ndarray, z_j: np.ndarray) -> np.ndarray:
    global LAST_RESULTS
    z_i = np.asarray(z_i, dtype=np.float32)
    z_j = np.asarray(z_j, dtype=np.float32)
    assert z_i.shape == (B, C) and z_j.shape == (B, C)

    reps = np.concatenate([z_i, z_j], axis=0).astype(ml_dtypes.bfloat16)
    in_maps = []
    for k in range(N_CORES):
        rot = np.roll(reps, -k * M_LOCAL, axis=0)
        tiled = np.ascontiguousarray(
            rot.reshape(NT, 128, C).transpose(1, 0, 2).reshape(128, NT * C)
        )
        in_maps.append({"keys": tiled})

    nc = _get_nc()
    trace = bool(int(os.environ.get("KERNEL_TRACE", "0")))
    res = run_bass_kernel_spmd(
        nc, in_maps, core_ids=list(range(N_CORES)), trace=trace
    )
    LAST_RESULTS = res
    total = sum(float(r["out"].sum()) for r in res.results)
    return np.float32(total / TWOB)
